# revision 2
# baseline (speedup 1.0000x reference)
"""Trainium2 Bass kernel for the AggregateLayer pooling problem.

reference semantics (per batch b):
    dot_w[j] = <pref[b,j,:], c[b,0,:]>                      (j = 0..63)
    t_w[j]   = 1 / |t_pref[b,0,j] - t_c[b,0]|
    w        = softmax(dot_w + t_w)                          (over j)
    u[b,0,:] = sum_j w[j] * pref[b,j,:]

Strategy: pure data parallel over 8 NeuronCores (1024 batches each),
batches in groups of GROUP=128 (64 two-batch tiles of 128 flattened
(batch, j) rows x 128 D cols).

Engine budget (measured): pref stream on the gpsimd SWDGE ring runs at
~712 GB/s (47 us); PE tiny matmuls (dot + weighted-sum, N=2) pipeline
their weight loads and cost ~36 ns each (37 us for all 1024); PE
transposes cost ~95 ns each; the XBAR dma-transpose (sync HWDGE) moves
a 128x128 fp16 tile in ~172 ns but is a single shared resource --
concurrent XPOSEs from two HWDGE queues corrupt each other's output
(measured), so ONLY the sync queue ever issues them, and XPOSE
destinations must be contiguous per partition.

The per-group transpose work (needed only to compute the dots) is
split between the XBAR (chunks 0..XCH-1) and PE+PSUM-copy (chunks
XCH..7), balancing PE ~57 us vs XBAR ~55 us. The pipeline runs the
transposes of group g+1 (both paths) during group g's dots/softmax/
weighted-sum, pref loads two groups ahead, and the weighted sum one
group behind, so no engine queue waits mid-stream. u is stored
transposed [D, BPC] straight from the weighted-sum PSUM layout; the
host transposes it back.
"""

import numpy as np
from contextlib import ExitStack

import concourse.bass as bass
import concourse.tile as tile
from concourse import mybir
from concourse.masks import make_identity
from concourse.bass_utils import run_bass_kernel_spmd
import concourse.bass2jax as _b2j


def _split_multiwait(bir: dict) -> int:
    """Walrus in this container rejects >1 sync-wait per instruction.

    Hoist excess waits onto NoOps inserted just before the instruction on
    the same engine (program order within the engine stream preserves the
    wait semantics exactly).
    """
    n = 0
    for fn in bir["functions"]:
        for blk in fn["blocks"]:
            out = []
            for inst in blk["instructions"]:
                si = inst.get("sync_info")
                waits = si.get("on_wait") if si else None
                if waits and len(waits) > 1:
                    for w in waits[:-1]:
                        out.append(
                            {
                                "opcode": "NoOp",
                                "engine": inst["engine"],
                                "name": f"{inst['name']}-xw{n}",
                                "ins": [],
                                "outs": [],
                                "sync_info": {"on_update": [], "on_wait": [w]},
                            }
                        )
                        n += 1
                    si["on_wait"] = [waits[-1]]
                out.append(inst)
            blk["instructions"] = out
    return n


_orig_compile_bir_kernel = _b2j.compile_bir_kernel


def _legalizing_compile_bir_kernel(ant_bir_str, *args, **kwargs):
    import orjson

    bir = orjson.loads(ant_bir_str)
    _split_multiwait(bir)
    return _orig_compile_bir_kernel(orjson.dumps(bir), *args, **kwargs)


_b2j.compile_bir_kernel = _legalizing_compile_bir_kernel

F32 = mybir.dt.float32
F16 = mybir.dt.float16
Alu = mybir.AluOpType
Act = mybir.ActivationFunctionType
Axis = mybir.AxisListType

B, N, D = 8192, 64, 128
NCORES = 8
BPC = B // NCORES          # 1024 batches per core
GROUP = 128                # batches per group
NGROUPS = BPC // GROUP     # 8
NTILES = GROUP // 2        # 64 two-batch tiles per group
NPAIR = GROUP // 2         # 64 batch-pairs per group (softmax partitions)
CH = 8                     # tiles per transpose/copy/dot chunk
NCH = NTILES // CH         # 8 chunks per group
XCH = 5                    # chunks transposed via XBAR (rest via PE)
HT = 16                    # tiles per pref DMA chunk


class _St:
    """Per-group pipeline state carried between build phases."""

    def __init__(self, g):
        self.g = g
        self.p16 = None
        self.pts = None
        self.tw = None
        self.w = None
        self.nmx = None
        self.wn16 = None
        self.wmat16 = None


class _Ctx:
    def __init__(self, tc, pools, consts, aps):
        self.tc = tc
        self.nc = tc.nc
        (self.p_p16, self.p_pts, self.p_small, self.ps_pt, self.ps_mm,
         self.ps_small) = pools
        self.ident16, self.ident32 = consts
        (self.pref_rows, self.u_all, self.ct16a, self.tpa, self.tca,
         self.cg16, self.wmat_ring, self.p16s, self.ptss) = aps


def _emit_pref_loads(cx, g, skip=0):
    """pref chunk DMAs for group g on the gpsimd SWDGE (fp32->fp16)."""
    nc = cx.nc
    r0 = g * GROUP * N
    for h0 in range(skip, NTILES, HT):
        rh = r0 + h0 * 128
        nc.gpsimd.dma_start(
            out=cx.p16s[g][:, h0 : h0 + HT, :],
            in_=cx.pref_rows[rh : rh + HT * 128, :].rearrange(
                "(t p) d -> p t d", p=128
            ),
        )


def _emit_xbar(cx, g):
    """XBAR transposes for chunks 0..XCH-1 of group g (sync queue ONLY).

    Destination slices are contiguous per partition (full-width innermost
    slices of the pts tile) -- required for correct XPOSE output.
    """
    nc = cx.nc
    for k in range(XCH):
        t0 = k * CH
        nc.sync.dma_start_transpose(
            out=cx.ptss[g][:, t0 : t0 + CH, :],
            in_=cx.p16s[g][:, t0 : t0 + CH, :].rearrange("p t d -> p (t d)"),
        )


def _emit_pe_T(cx, g):
    """PE transposes for chunks XCH..NCH-1 of group g + PSUM->SBUF copies."""
    nc = cx.nc
    for k in range(XCH, NCH):
        t0 = k * CH
        pt_ps = cx.ps_pt.tile(
            [128, CH, 128], F16, tag="pt_ps", name=f"ptps{g}_{k}"
        )
        for i in range(CH):
            nc.tensor.transpose(
                out=pt_ps[:, i, :],
                in_=cx.p16s[g][:, t0 + i, :],
                identity=cx.ident16[:],
            )
        if k == XCH:
            nc.vector.tensor_copy(
                out=cx.ptss[g][:, t0 : t0 + CH, :], in_=pt_ps[:]
            )
        else:
            nc.scalar.copy(out=cx.ptss[g][:, t0 : t0 + CH, :], in_=pt_ps[:])


def _phase_dots(cx, g):
    """tw + dot matmuls from pts + extraction + add/max + softmax tail."""
    nc = cx.nc
    st = _St(g)
    st.p16 = cx.p16s[g]
    st.pts = cx.ptss[g]

    st.tw = cx.p_small.tile([NPAIR, 2, N], F32, tag="tw", name=f"tw{g}")
    for s in range(2):
        nc.vector.tensor_scalar_sub(
            out=st.tw[:, s, :],
            in0=cx.tpa[:, g, s, :],
            scalar1=cx.tca[:, g, s : s + 1],
        )
    nc.scalar.activation(out=st.tw[:], in_=st.tw[:], func=Act.Abs)
    nc.vector.reciprocal(out=st.tw[:], in_=st.tw[:])

    ps_dots = cx.ps_mm.tile(
        [128, NTILES, 2], F32, tag="mm_ps", name=f"dots{g}"
    )
    for k in range(NCH):
        t0 = k * CH
        for i in range(CH):
            t = t0 + i
            nc.tensor.matmul(
                out=ps_dots[:, t, :],
                lhsT=st.pts[:, t, :],
                rhs=cx.ct16a[:, g, 2 * t : 2 * t + 2],
                start=(i == 0),
                stop=(i == CH - 1),
            )

    # valid dots sit at [row, parity=row//64]: extract the two halves
    dotw = cx.p_small.tile([128, NTILES], F32, tag="dotw", name=f"dotw{g}")
    nc.scalar.copy(out=dotw[0:64, :], in_=ps_dots[0:64, :, 0])
    nc.scalar.copy(out=dotw[64:128, :], in_=ps_dots[64:128, :, 1])

    # transpose [128(row), nt] -> [nt, 128(row)] => pair-major dots
    dr_ps = cx.ps_small.tile([NPAIR, 128], F32, tag="sm_ps", name=f"dr{g}")
    nc.tensor.transpose(out=dr_ps[:], in_=dotw[:], identity=cx.ident32[:])

    st.w = cx.p_small.tile([NPAIR, 2, N], F32, tag="w", name=f"w{g}")
    nc.vector.tensor_add(
        out=st.w[:],
        in0=dr_ps[:].rearrange("t (two n) -> t two n", two=2),
        in1=st.tw[:],
    )
    st.nmx = cx.p_small.tile([NPAIR, 2], F32, tag="nmx", name=f"nmx{g}")
    nc.vector.tensor_reduce(
        out=st.nmx[:], in_=st.w[:], axis=Axis.X, op=Alu.max, negate=True
    )
    _phase_b1(cx, st)
    return st


def _phase_b1(cx, st):
    """Softmax tail: exp + sum + reciprocal + normalize (no PE)."""
    nc = cx.nc
    g = st.g
    e = cx.p_small.tile([NPAIR, 2, N], F32, tag="e", name=f"e{g}")
    for s in range(2):
        nc.scalar.activation(
            out=e[:, s, :],
            in_=st.w[:, s, :],
            func=Act.Exp,
            bias=st.nmx[:, s : s + 1],
            scale=1.0,
        )
    z = cx.p_small.tile([NPAIR, 2], F32, tag="z", name=f"z{g}")
    nc.vector.reduce_sum(out=z[:], in_=e[:], axis=Axis.X)
    rz = cx.p_small.tile([NPAIR, 2], F32, tag="rz", name=f"rz{g}")
    nc.vector.reciprocal(out=rz[:], in_=z[:])
    st.wn16 = cx.p_small.tile([NPAIR, 2, N], F16, tag="wn16", name=f"wn{g}")
    for s in range(2):
        nc.vector.tensor_scalar_mul(
            out=st.wn16[:, s, :], in0=e[:, s, :], scalar1=rz[:, s : s + 1]
        )


def _phase_b2(cx, st):
    """W_MAT build: PE transpose of wn16 + block scatter."""
    nc = cx.nc
    g = st.g
    wc_ps = cx.ps_small.tile([128, NTILES], F16, tag="sm_ps", name=f"wc{g}")
    nc.tensor.transpose(
        out=wc_ps[:],
        in_=st.wn16[:].rearrange("t two n -> t (two n)"),
        identity=cx.ident16[0:NPAIR, 0:NPAIR],
    )
    wcol = cx.p_small.tile([128, NTILES], F16, tag="wcol", name=f"wcol{g}")
    nc.vector.tensor_copy(out=wcol[:], in_=wc_ps[:])
    # persistent pre-zeroed ring: only the data halves are ever written,
    # the zero halves survive across generations
    st.wmat16 = cx.wmat_ring[g % len(cx.wmat_ring)]
    nc.vector.tensor_copy(out=st.wmat16[0:64, :, 0], in_=wcol[0:64, :])
    nc.vector.tensor_copy(out=st.wmat16[64:128, :, 1], in_=wcol[64:128, :])


def _phase_c(cx, st):
    """Weighted-sum matmuls + u extraction (DVE) + store (gpsimd)."""
    nc = cx.nc
    g = st.g
    b0 = g * GROUP
    HB = NTILES // 2
    for h in range(2):
        ps_ut = cx.ps_mm.tile(
            [128, HB, 2], F32, tag="mm_ps", name=f"ut{g}_{h}"
        )
        for k in range(HB):
            t = h * HB + k
            nc.tensor.matmul(
                out=ps_ut[:, k, :],
                lhsT=st.p16[:, t, :],
                rhs=st.wmat16[:, t, :],
                start=(k == 0),
                stop=(k == HB - 1),
            )
        uts = cx.p_small.tile(
            [128, GROUP // 2], F32, tag="uts", name=f"uts{g}_{h}"
        )
        nc.vector.tensor_copy(
            out=uts[:], in_=ps_ut[:].rearrange("d t two -> d (t two)")
        )
        bh = b0 + h * (GROUP // 2)
        nc.gpsimd.dma_start(
            out=cx.u_all[:, bh : bh + GROUP // 2], in_=uts[:]
        )


def _emit_ct(cx, g):
    """PE transpose of group g's c half into ct16a (+ DVE copy)."""
    nc = cx.nc
    ct_ps = cx.ps_small.tile([128, 128], F16, tag="sm_ps", name=f"ct{g}")
    nc.tensor.transpose(
        out=ct_ps[:],
        in_=cx.cg16[:, g, :],
        identity=cx.ident16[:],
    )
    nc.vector.tensor_copy(out=cx.ct16a[:, g, :], in_=ct_ps[:])


def _build_nc():
    nc = bass.Bass()
    pref = nc.declare_dram_parameter("pref", [BPC, N, D], F32, isOutput=False)
    c = nc.declare_dram_parameter("c", [BPC, 1, D], F32, isOutput=False)
    t_pref = nc.declare_dram_parameter("t_pref", [BPC, 1, N], F32, isOutput=False)
    t_c = nc.declare_dram_parameter("t_c", [BPC, 1], F32, isOutput=False)
    # u stored transposed [D, BPC] (direct from the weighted-sum PSUM
    # layout); the host transposes it back.
    u = nc.declare_dram_parameter("u", [D, BPC], F32, isOutput=True)

    pref_rows = pref[:].rearrange("b n d -> (b n) d")
    c_all = c[:].rearrange("b one d -> (b one) d")
    tp_all = t_pref[:].rearrange("b one n -> (b one) n")
    tc_all = t_c[:]
    u_all = u[:]

    with ExitStack() as ctx:
        tc = ctx.enter_context(tile.TileContext(nc))
        p_const = ctx.enter_context(tc.tile_pool(name="const", bufs=1))
        p_pre = ctx.enter_context(tc.tile_pool(name="pre", bufs=1))
        p_p16 = ctx.enter_context(tc.tile_pool(name="p16", bufs=4))
        p_pts = ctx.enter_context(tc.tile_pool(name="pts", bufs=3))
        p_small = ctx.enter_context(tc.tile_pool(name="small", bufs=3))
        ps_pt = ctx.enter_context(tc.tile_pool(name="ps_pt", bufs=3, space="PSUM"))
        ps_mm = ctx.enter_context(tc.tile_pool(name="ps_mm", bufs=3, space="PSUM"))
        ps_small = ctx.enter_context(
            tc.tile_pool(name="ps_small", bufs=2, space="PSUM")
        )

        nb = NGROUPS * GROUP

        # persistent rings
        p16s = []
        ptss = []
        for _gi in range(NGROUPS):
            p16s.append(
                p_p16.tile([128, NTILES, D], F16, tag="p16", name=f"p16_{_gi}")
            )
            ptss.append(
                p_pts.tile([128, NTILES, D], F16, tag="pts", name=f"pts_{_gi}")
            )

        # gpsimd stream head: group 0's first pref chunk, then identity
        # builds (gpsimd-only affine_select), then c/t loads, then the
        # rest of pref.
        nc.gpsimd.dma_start(
            out=p16s[0][:, 0:HT, :],
            in_=pref_rows[0 : HT * 128, :].rearrange("(t p) d -> p t d", p=128),
        )
        ident16 = p_const.tile([128, 128], F16)
        make_identity(nc, ident16[:])

        c32a = p_pre.tile([128, NGROUPS, D], F32)
        nc.gpsimd.dma_start(
            out=c32a[:],
            in_=c_all[0:nb, :].rearrange("(g b) d -> b g d", b=128),
        )
        tpa = p_pre.tile([NPAIR, NGROUPS, 2, N], F32)
        nc.gpsimd.dma_start(
            out=tpa[:],
            in_=tp_all[0:nb, :].rearrange(
                "(g t two) n -> t g two n", t=NPAIR, two=2
            ),
        )
        tca = p_pre.tile([NPAIR, NGROUPS, 2], F32)
        nc.gpsimd.dma_start(
            out=tca[:],
            in_=tc_all[0:nb, :].rearrange(
                "(g t two) one -> t g (two one)", t=NPAIR, two=2
            ),
        )
        ident32 = p_const.tile([128, 128], F32)
        make_identity(nc, ident32[:])
        consts = (ident16, ident32)

        _emit_pref_loads_head = True
        # rest of group 0 + group 1 pref
        # (emitted below via cx; build cx first)
        cg16 = p_pre.tile([128, NGROUPS, D], F16)
        ct16a = p_pre.tile([128, NGROUPS, 128], F16)  # [D, group, batch]

        wmat_ring = []
        for _wi in range(3):
            wm = p_pre.tile([128, NTILES, 2], F16, name=f"wmatr{_wi}")
            nc.vector.memset(wm[:], 0.0)
            wmat_ring.append(wm)

        aps = (pref_rows, u_all, ct16a, tpa, tca, cg16, wmat_ring, p16s, ptss)
        cx = _Ctx(tc, (p_p16, p_pts, p_small, ps_pt, ps_mm, ps_small),
                  consts, aps)

        _emit_pref_loads(cx, 0, skip=HT)
        _emit_pref_loads(cx, 1)

        # c cast (DVE) + group-0/1 prep
        nc.vector.tensor_copy(out=cg16[:], in_=c32a[:])
        _emit_ct(cx, 0)
        _emit_xbar(cx, 0)
        _emit_pe_T(cx, 0)
        _emit_ct(cx, 1)

        # software pipeline, coarse blocks:
        #   iter g: [pref g+2] [xbar g+1] [pe-T g+1] [dots+softmax g]
        #           [wsum g-1] [W_MAT g] [cT g+2]
        pend = None
        for g in range(NGROUPS):
            if g + 2 < NGROUPS:
                _emit_pref_loads(cx, g + 2)
            if g + 1 < NGROUPS:
                _emit_xbar(cx, g + 1)
                _emit_pe_T(cx, g + 1)
            st = _phase_dots(cx, g)
            if pend is not None:
                _phase_c(cx, pend)
            _phase_b2(cx, st)
            if g + 2 < NGROUPS:
                _emit_ct(cx, g + 2)
            pend = st

        _phase_c(cx, pend)

    return nc


_NC_CACHE = None
LAST_RESULT = None


def kernel(pref, c, t_pref, t_c):
    global _NC_CACHE, LAST_RESULT
    if _NC_CACHE is None:
        _NC_CACHE = _build_nc()
    nc = _NC_CACHE

    pref = np.ascontiguousarray(pref, dtype=np.float32)
    c = np.ascontiguousarray(c, dtype=np.float32)
    t_pref = np.ascontiguousarray(t_pref, dtype=np.float32)
    t_c = np.ascontiguousarray(t_c, dtype=np.float32)

    in_maps = []
    for i in range(NCORES):
        s = slice(i * BPC, (i + 1) * BPC)
        in_maps.append(
            {"pref": pref[s], "c": c[s], "t_pref": t_pref[s], "t_c": t_c[s]}
        )

    res = run_bass_kernel_spmd(nc, in_maps, list(range(NCORES)))
    LAST_RESULT = res
    return np.ascontiguousarray(
        np.concatenate([r["u"].T for r in res.results], axis=0)
    ).reshape(B, 1, D)


# revision 3
# speedup vs baseline: 1.3957x; 1.3957x over previous
"""Trainium2 Bass kernel for the AggregateLayer pooling problem.

reference semantics (per batch b):
    dot_w[j] = <pref[b,j,:], c[b,0,:]>                      (j = 0..63)
    t_w[j]   = 1 / |t_pref[b,0,j] - t_c[b,0]|
    w        = softmax(dot_w + t_w)                          (over j)
    u[b,0,:] = sum_j w[j] * pref[b,j,:]

Strategy: pure data parallel over 8 NeuronCores (1024 batches each),
batches in groups of GROUP=128 (64 two-batch tiles of 128 flattened
(batch, j) rows x 128 D cols).

Engine budget (measured): pref stream on the gpsimd SWDGE ring runs at
~712 GB/s (47 us); PE tiny matmuls (dot + weighted-sum, N=2) pipeline
their weight loads and cost ~36 ns each (37 us for all 1024); PE
transposes cost ~95 ns each; the XBAR dma-transpose (sync HWDGE) moves
a 128x128 fp16 tile in ~172 ns but is a single shared resource --
concurrent XPOSEs from two HWDGE queues corrupt each other's output
(measured), so ONLY the sync queue ever issues them, and XPOSE
destinations must be contiguous per partition.

The per-group transpose work (needed only to compute the dots) is
split between the XBAR (chunks 0..XCH-1) and PE+PSUM-copy (chunks
XCH..7), balancing PE ~57 us vs XBAR ~55 us. The pipeline runs the
transposes of group g+1 (both paths) during group g's dots/softmax/
weighted-sum, pref loads two groups ahead, and the weighted sum one
group behind, so no engine queue waits mid-stream. u is stored
transposed [D, BPC] straight from the weighted-sum PSUM layout; the
host transposes it back.
"""

import numpy as np
from contextlib import ExitStack

import concourse.bass as bass
import concourse.tile as tile
from concourse import mybir
from concourse.masks import make_identity
from concourse.bass_utils import run_bass_kernel_spmd
import concourse.bass2jax as _b2j


def _split_multiwait(bir: dict) -> int:
    """Walrus in this container rejects >1 sync-wait per instruction.

    Hoist excess waits onto NoOps inserted just before the instruction on
    the same engine (program order within the engine stream preserves the
    wait semantics exactly).
    """
    n = 0
    for fn in bir["functions"]:
        for blk in fn["blocks"]:
            out = []
            for inst in blk["instructions"]:
                si = inst.get("sync_info")
                waits = si.get("on_wait") if si else None
                if waits and len(waits) > 1:
                    for w in waits[:-1]:
                        out.append(
                            {
                                "opcode": "NoOp",
                                "engine": inst["engine"],
                                "name": f"{inst['name']}-xw{n}",
                                "ins": [],
                                "outs": [],
                                "sync_info": {"on_update": [], "on_wait": [w]},
                            }
                        )
                        n += 1
                    si["on_wait"] = [waits[-1]]
                out.append(inst)
            blk["instructions"] = out
    return n


_orig_compile_bir_kernel = _b2j.compile_bir_kernel


def _legalizing_compile_bir_kernel(ant_bir_str, *args, **kwargs):
    import orjson

    bir = orjson.loads(ant_bir_str)
    _split_multiwait(bir)
    return _orig_compile_bir_kernel(orjson.dumps(bir), *args, **kwargs)


_b2j.compile_bir_kernel = _legalizing_compile_bir_kernel

F32 = mybir.dt.float32
F16 = mybir.dt.float16
Alu = mybir.AluOpType
Act = mybir.ActivationFunctionType
Axis = mybir.AxisListType

B, N, D = 8192, 64, 128
NCORES = 8
BPC = B // NCORES          # 1024 batches per core
GROUP = 128                # batches per group
NGROUPS = BPC // GROUP     # 8
NTILES = GROUP // 2        # 64 two-batch tiles per group
NPAIR = GROUP // 2         # 64 batch-pairs per group (softmax partitions)
CH = 8                     # tiles per transpose/copy/dot chunk
NCH = NTILES // CH         # 8 chunks per group
XCH = 5                    # chunks transposed via XBAR (rest via PE)
HT = 16                    # tiles per pref DMA chunk


class _St:
    """Per-group pipeline state carried between build phases."""

    def __init__(self, g):
        self.g = g
        self.p16 = None
        self.pts = None
        self.tw = None
        self.w = None
        self.nmx = None
        self.wn16 = None
        self.wmat16 = None


class _Ctx:
    def __init__(self, tc, pools, consts, aps):
        self.tc = tc
        self.nc = tc.nc
        (self.p_p16, self.p_pts, self.p_small, self.ps_pt, self.ps_mm,
         self.ps_small) = pools
        self.ident16, self.ident32 = consts
        (self.pref_rows, self.u_all, self.ct16a, self.tpa, self.tca,
         self.cg16, self.wmat_ring, self.p16s, self.ptss) = aps


def _emit_pref_loads(cx, g, skip=0):
    """pref chunk DMAs for group g on the gpsimd SWDGE (fp32->fp16)."""
    nc = cx.nc
    r0 = g * GROUP * N
    for h0 in range(skip, NTILES, HT):
        rh = r0 + h0 * 128
        nc.gpsimd.dma_start(
            out=cx.p16s[g][:, h0 : h0 + HT, :],
            in_=cx.pref_rows[rh : rh + HT * 128, :].rearrange(
                "(t p) d -> p t d", p=128
            ),
        )


def _emit_xbar(cx, g):
    """XBAR transposes for chunks 0..XCH-1 of group g (sync queue ONLY).

    Destination slices are contiguous per partition (full-width innermost
    slices of the pts tile) -- required for correct XPOSE output. One big
    XPOSE per group: fewer HWDGE instructions entangle less with the
    SWDGE vector-clock waits (each extra XPOSE adds spurious cross-ring
    ordering edges on later gpsimd loads).
    """
    nc = cx.nc
    nt = XCH * CH
    nc.sync.dma_start_transpose(
        out=cx.ptss[g][:, 0:nt, :],
        in_=cx.p16s[g][:, 0:nt, :].rearrange("p t d -> p (t d)"),
    )


def _emit_pe_T(cx, g):
    """PE transposes for chunks XCH..NCH-1 of group g + PSUM->SBUF copies."""
    nc = cx.nc
    for k in range(XCH, NCH):
        t0 = k * CH
        pt_ps = cx.ps_pt.tile(
            [128, CH, 128], F16, tag="pt_ps", name=f"ptps{g}_{k}"
        )
        for i in range(CH):
            nc.tensor.transpose(
                out=pt_ps[:, i, :],
                in_=cx.p16s[g][:, t0 + i, :],
                identity=cx.ident16[:],
            )
        if k == XCH:
            nc.vector.tensor_copy(
                out=cx.ptss[g][:, t0 : t0 + CH, :], in_=pt_ps[:]
            )
        else:
            nc.scalar.copy(out=cx.ptss[g][:, t0 : t0 + CH, :], in_=pt_ps[:])


def _phase_dots(cx, g):
    """tw + dot matmuls from pts + extraction + add/max + softmax tail."""
    nc = cx.nc
    st = _St(g)
    st.p16 = cx.p16s[g]
    st.pts = cx.ptss[g]

    st.tw = cx.p_small.tile([NPAIR, 2, N], F32, tag="tw", name=f"tw{g}")
    for s in range(2):
        nc.vector.tensor_scalar_sub(
            out=st.tw[:, s, :],
            in0=cx.tpa[:, g, s, :],
            scalar1=cx.tca[:, g, s : s + 1],
        )
    nc.scalar.activation(out=st.tw[:], in_=st.tw[:], func=Act.Abs)
    nc.vector.reciprocal(out=st.tw[:], in_=st.tw[:])

    ps_dots = cx.ps_mm.tile(
        [128, NTILES, 2], F32, tag="mm_ps", name=f"dots{g}"
    )
    for k in range(NCH):
        t0 = k * CH
        for i in range(CH):
            t = t0 + i
            nc.tensor.matmul(
                out=ps_dots[:, t, :],
                lhsT=st.pts[:, t, :],
                rhs=cx.ct16a[:, g, 2 * t : 2 * t + 2],
                start=(i == 0),
                stop=(i == CH - 1),
            )

    # valid dots sit at [row, parity=row//64]: extract the two halves
    dotw = cx.p_small.tile([128, NTILES], F32, tag="dotw", name=f"dotw{g}")
    nc.scalar.copy(out=dotw[0:64, :], in_=ps_dots[0:64, :, 0])
    nc.scalar.copy(out=dotw[64:128, :], in_=ps_dots[64:128, :, 1])

    # transpose [128(row), nt] -> [nt, 128(row)] => pair-major dots
    dr_ps = cx.ps_small.tile([NPAIR, 128], F32, tag="sm_ps", name=f"dr{g}")
    nc.tensor.transpose(out=dr_ps[:], in_=dotw[:], identity=cx.ident32[:])

    st.w = cx.p_small.tile([NPAIR, 2, N], F32, tag="w", name=f"w{g}")
    nc.vector.tensor_add(
        out=st.w[:],
        in0=dr_ps[:].rearrange("t (two n) -> t two n", two=2),
        in1=st.tw[:],
    )
    st.nmx = cx.p_small.tile([NPAIR, 2], F32, tag="nmx", name=f"nmx{g}")
    nc.vector.tensor_reduce(
        out=st.nmx[:], in_=st.w[:], axis=Axis.X, op=Alu.max, negate=True
    )
    _phase_b1(cx, st)
    return st


def _phase_b1(cx, st):
    """Softmax tail: exp + sum + reciprocal + normalize (no PE)."""
    nc = cx.nc
    g = st.g
    e = cx.p_small.tile([NPAIR, 2, N], F32, tag="e", name=f"e{g}")
    for s in range(2):
        nc.scalar.activation(
            out=e[:, s, :],
            in_=st.w[:, s, :],
            func=Act.Exp,
            bias=st.nmx[:, s : s + 1],
            scale=1.0,
        )
    z = cx.p_small.tile([NPAIR, 2], F32, tag="z", name=f"z{g}")
    nc.vector.reduce_sum(out=z[:], in_=e[:], axis=Axis.X)
    rz = cx.p_small.tile([NPAIR, 2], F32, tag="rz", name=f"rz{g}")
    nc.vector.reciprocal(out=rz[:], in_=z[:])
    st.wn16 = cx.p_small.tile([NPAIR, 2, N], F16, tag="wn16", name=f"wn{g}")
    for s in range(2):
        nc.vector.tensor_scalar_mul(
            out=st.wn16[:, s, :], in0=e[:, s, :], scalar1=rz[:, s : s + 1]
        )


def _phase_b2(cx, st):
    """W_MAT build: PE transpose of wn16 + block scatter."""
    nc = cx.nc
    g = st.g
    wc_ps = cx.ps_small.tile([128, NTILES], F16, tag="sm_ps", name=f"wc{g}")
    nc.tensor.transpose(
        out=wc_ps[:],
        in_=st.wn16[:].rearrange("t two n -> t (two n)"),
        identity=cx.ident16[0:NPAIR, 0:NPAIR],
    )
    wcol = cx.p_small.tile([128, NTILES], F16, tag="wcol", name=f"wcol{g}")
    nc.vector.tensor_copy(out=wcol[:], in_=wc_ps[:])
    # persistent pre-zeroed ring: only the data halves are ever written,
    # the zero halves survive across generations
    st.wmat16 = cx.wmat_ring[g % len(cx.wmat_ring)]
    nc.vector.tensor_copy(out=st.wmat16[0:64, :, 0], in_=wcol[0:64, :])
    nc.vector.tensor_copy(out=st.wmat16[64:128, :, 1], in_=wcol[64:128, :])


def _phase_c(cx, st):
    """Weighted-sum matmuls + u extraction (DVE) + store (gpsimd)."""
    nc = cx.nc
    g = st.g
    b0 = g * GROUP
    HB = NTILES // 2
    for h in range(2):
        ps_ut = cx.ps_mm.tile(
            [128, HB, 2], F32, tag="mm_ps", name=f"ut{g}_{h}"
        )
        for k in range(HB):
            t = h * HB + k
            nc.tensor.matmul(
                out=ps_ut[:, k, :],
                lhsT=st.p16[:, t, :],
                rhs=st.wmat16[:, t, :],
                start=(k == 0),
                stop=(k == HB - 1),
            )
        uts = cx.p_small.tile(
            [128, GROUP // 2], F32, tag="uts", name=f"uts{g}_{h}"
        )
        nc.vector.tensor_copy(
            out=uts[:], in_=ps_ut[:].rearrange("d t two -> d (t two)")
        )
        bh = b0 + h * (GROUP // 2)
        nc.gpsimd.dma_start(
            out=cx.u_all[:, bh : bh + GROUP // 2], in_=uts[:]
        )


def _emit_ct(cx, g):
    """PE transpose of group g's c half into ct16a (+ DVE copy)."""
    nc = cx.nc
    ct_ps = cx.ps_small.tile([128, 128], F16, tag="sm_ps", name=f"ct{g}")
    nc.tensor.transpose(
        out=ct_ps[:],
        in_=cx.cg16[:, g, :],
        identity=cx.ident16[:],
    )
    nc.vector.tensor_copy(out=cx.ct16a[:, g, :], in_=ct_ps[:])


def _build_nc():
    nc = bass.Bass()
    pref = nc.declare_dram_parameter("pref", [BPC, N, D], F32, isOutput=False)
    c = nc.declare_dram_parameter("c", [BPC, 1, D], F32, isOutput=False)
    t_pref = nc.declare_dram_parameter("t_pref", [BPC, 1, N], F32, isOutput=False)
    t_c = nc.declare_dram_parameter("t_c", [BPC, 1], F32, isOutput=False)
    # u stored transposed [D, BPC] (direct from the weighted-sum PSUM
    # layout); the host transposes it back.
    u = nc.declare_dram_parameter("u", [D, BPC], F32, isOutput=True)

    pref_rows = pref[:].rearrange("b n d -> (b n) d")
    c_all = c[:].rearrange("b one d -> (b one) d")
    tp_all = t_pref[:].rearrange("b one n -> (b one) n")
    tc_all = t_c[:]
    u_all = u[:]

    with ExitStack() as ctx:
        tc = ctx.enter_context(tile.TileContext(nc))
        p_const = ctx.enter_context(tc.tile_pool(name="const", bufs=1))
        p_pre = ctx.enter_context(tc.tile_pool(name="pre", bufs=1))
        p_p16 = ctx.enter_context(tc.tile_pool(name="p16", bufs=4))
        p_pts = ctx.enter_context(tc.tile_pool(name="pts", bufs=3))
        p_small = ctx.enter_context(tc.tile_pool(name="small", bufs=3))
        ps_pt = ctx.enter_context(tc.tile_pool(name="ps_pt", bufs=3, space="PSUM"))
        ps_mm = ctx.enter_context(tc.tile_pool(name="ps_mm", bufs=3, space="PSUM"))
        ps_small = ctx.enter_context(
            tc.tile_pool(name="ps_small", bufs=2, space="PSUM")
        )

        nb = NGROUPS * GROUP

        # persistent rings
        p16s = []
        ptss = []
        for _gi in range(NGROUPS):
            p16s.append(
                p_p16.tile([128, NTILES, D], F16, tag="p16", name=f"p16_{_gi}")
            )
            ptss.append(
                p_pts.tile([128, NTILES, D], F16, tag="pts", name=f"pts_{_gi}")
            )

        # gpsimd stream head: group 0's first pref chunk, then identity
        # builds (gpsimd-only affine_select), then c/t loads, then the
        # rest of pref.
        nc.gpsimd.dma_start(
            out=p16s[0][:, 0:HT, :],
            in_=pref_rows[0 : HT * 128, :].rearrange("(t p) d -> p t d", p=128),
        )
        ident16 = p_const.tile([128, 128], F16)
        make_identity(nc, ident16[:])

        c32a = p_pre.tile([128, NGROUPS, D], F32)
        nc.gpsimd.dma_start(
            out=c32a[:],
            in_=c_all[0:nb, :].rearrange("(g b) d -> b g d", b=128),
        )
        tpa = p_pre.tile([NPAIR, NGROUPS, 2, N], F32)
        nc.gpsimd.dma_start(
            out=tpa[:],
            in_=tp_all[0:nb, :].rearrange(
                "(g t two) n -> t g two n", t=NPAIR, two=2
            ),
        )
        tca = p_pre.tile([NPAIR, NGROUPS, 2], F32)
        nc.gpsimd.dma_start(
            out=tca[:],
            in_=tc_all[0:nb, :].rearrange(
                "(g t two) one -> t g (two one)", t=NPAIR, two=2
            ),
        )
        ident32 = p_const.tile([128, 128], F32)
        make_identity(nc, ident32[:])
        consts = (ident16, ident32)

        _emit_pref_loads_head = True
        # rest of group 0 + group 1 pref
        # (emitted below via cx; build cx first)
        cg16 = p_pre.tile([128, NGROUPS, D], F16)
        ct16a = p_pre.tile([128, NGROUPS, 128], F16)  # [D, group, batch]

        wmat_ring = []
        for _wi in range(3):
            wm = p_pre.tile([128, NTILES, 2], F16, name=f"wmatr{_wi}")
            nc.vector.memset(wm[:], 0.0)
            wmat_ring.append(wm)

        aps = (pref_rows, u_all, ct16a, tpa, tca, cg16, wmat_ring, p16s, ptss)
        cx = _Ctx(tc, (p_p16, p_pts, p_small, ps_pt, ps_mm, ps_small),
                  consts, aps)

        _emit_pref_loads(cx, 0, skip=HT)
        _emit_pref_loads(cx, 1)

        # c cast (DVE) + group-0/1 prep
        nc.vector.tensor_copy(out=cg16[:], in_=c32a[:])
        _emit_ct(cx, 0)
        _emit_xbar(cx, 0)
        _emit_pe_T(cx, 0)
        _emit_ct(cx, 1)

        # software pipeline, coarse blocks:
        #   iter g: [pref g+2] [xbar g+1] [pe-T g+1] [dots+softmax g]
        #           [wsum g-1] [W_MAT g] [cT g+2]
        pend = None
        for g in range(NGROUPS):
            if g + 2 < NGROUPS:
                _emit_pref_loads(cx, g + 2)
            if g + 1 < NGROUPS:
                _emit_xbar(cx, g + 1)
                _emit_pe_T(cx, g + 1)
            st = _phase_dots(cx, g)
            if pend is not None:
                _phase_c(cx, pend)
            _phase_b2(cx, st)
            if g + 2 < NGROUPS:
                _emit_ct(cx, g + 2)
            pend = st

        _phase_c(cx, pend)

    return nc


_NC_CACHE = None
LAST_RESULT = None


def kernel(pref, c, t_pref, t_c):
    global _NC_CACHE, LAST_RESULT
    if _NC_CACHE is None:
        _NC_CACHE = _build_nc()
    nc = _NC_CACHE

    pref = np.ascontiguousarray(pref, dtype=np.float32)
    c = np.ascontiguousarray(c, dtype=np.float32)
    t_pref = np.ascontiguousarray(t_pref, dtype=np.float32)
    t_c = np.ascontiguousarray(t_c, dtype=np.float32)

    in_maps = []
    for i in range(NCORES):
        s = slice(i * BPC, (i + 1) * BPC)
        in_maps.append(
            {"pref": pref[s], "c": c[s], "t_pref": t_pref[s], "t_c": t_c[s]}
        )

    res = run_bass_kernel_spmd(nc, in_maps, list(range(NCORES)))
    LAST_RESULT = res
    return np.ascontiguousarray(
        np.concatenate([r["u"].T for r in res.results], axis=0)
    ).reshape(B, 1, D)
